# revision 6
# baseline (speedup 1.0000x reference)
"""DepLabeledGCN Trainium2 kernel — data-parallel variant (no collectives).

Each core processes ITS OWN batch with ALL 48 label matrices:
    s-phase:  sT[kc,l] chunks = per-label masked-adjacency matmuls (fp16,
              masks exact 0/1), label PAIRS fused into N=256 matmuls
    msum:     msg = sum_{l,kc} sT[kc,l] @ W_l^T[kc], 192 accumulating
              matmuls into one PSUM bank per layer
    relu(msg * 1/denom) -> next layer h (4 kc-chunk DVE ops)
then the 2-layer MLP (PE-transpose + packed PSUM) on the same core.

v2 scheduling changes vs baseline:
  - all small inputs packed into ONE dram tensor / one DMA
  - masks for pair 0 + chunked h0 cast first => first matmul ~2.5us earlier
  - sT tile keeps the PSUM layout [q,kc,l,i]; psum->sbuf copy is two
    contiguous halves on different engines (vector/scalar/gpsimd)
  - weight DMAs alternate sync/gpsimd queues (2 DGE streams)
  - relu + h0 casts split into kc chunks to cut layer-boundary latency
  - 16 pair-granular wres loads, 1 MLP-weight load, 1 output DMA
    (fewer queues => shorter semaphore-reset teardown)
"""

import sys

if '/opt/trn_rl_repo' not in sys.path:
    sys.path.insert(0, '/opt/trn_rl_repo')

import numpy as np

B, N, D, L = 8, 128, 512, 48
NCORES = 8
KC = D // 128
NUM_LAYERS = 2
R_RES = 32              # labels kept resident for layer 2
NP = L // 2             # label pairs per layer

# pack column offsets (fp32 units)
P_GCN = 0
P_ADJT = 512
P_LABT = 640
P_ADJR = 768
P_B0 = 896
P_B1 = 900
P_TOT = 904

_CACHE = {}


def _build_nc():
    import concourse.bass as bass
    import concourse.mybir as mybir
    import concourse.tile as tile
    from concourse import bacc
    from concourse.masks import make_identity

    dt = mybir.dt
    f32 = dt.float32
    f16 = dt.float16
    Alu = mybir.AluOpType

    nc = bacc.Bacc("TRN2", target_bir_lowering=False, debug=False,
                   num_devices=NCORES)

    pack_e = nc.dram_tensor("pack", [N, P_TOT], f32, kind="ExternalInput").ap()
    wT_e = nc.dram_tensor("wT", [128, L, KC, D], f16, kind="ExternalInput").ap()
    mlpw_e = nc.dram_tensor("mlpw", [128, 2, KC, D], f16,
                            kind="ExternalInput").ap()
    out_e = nc.dram_tensor("out", [128, KC, 128], f32,
                           kind="ExternalOutput").ap()

    with tile.TileContext(nc) as tc:
        with (
            tc.tile_pool(name="const", bufs=1) as cpool,
            tc.tile_pool(name="sT", bufs=3) as sT_pool,
            tc.tile_pool(name="wst", bufs=4) as wst_pool,
            tc.tile_pool(name="spsum", bufs=3, space="PSUM") as spsum,
            tc.tile_pool(name="mpsum", bufs=2, space="PSUM") as mpsum,
        ):
            # -------- critical-path input load (single DMA) -----------------
            pack_sb = cpool.tile([128, P_TOT], f32, tag="pack")
            nc.sync.dma_start(pack_sb[:], pack_e)
            gcn_v = pack_sb[:, P_GCN:P_GCN + D]
            adjT_v = pack_sb[:, P_ADJT:P_ADJT + N]
            labT_v = pack_sb[:, P_LABT:P_LABT + N]
            adjR_v = pack_sb[:, P_ADJR:P_ADJR + N]
            b0_v = pack_sb[:, P_B0:P_B0 + KC]
            b1_v = pack_sb[:, P_B1:P_B1 + KC]

            # resident weights, pair-granular, alternating DMA queues
            wres = cpool.tile([128, R_RES, KC, D], f16, tag="wres")
            for p in range(R_RES // 2):
                eng = nc.sync if p % 2 == 0 else nc.gpsimd
                eng.dma_start(wres[:, 2 * p:2 * p + 2], wT_e[:, 2 * p:2 * p + 2])

            h = [cpool.tile([128, D], f16, tag=f"h{ly}", name=f"h{ly}")
                 for ly in range(NUM_LAYERS + 1)]

            # -------- masks: maskT[j, l, i] = (labT == l) * adjT ------------
            maskT = cpool.tile([128, L, N], f16, tag="maskT")

            def emit_mask(l):
                nc.vector.scalar_tensor_tensor(
                    out=maskT[:, l, :],
                    in0=labT_v,
                    scalar=float(l),
                    in1=adjT_v,
                    op0=Alu.is_equal,
                    op1=Alu.mult,
                )

            # pair-0 masks first, then h0 chunks (vector/scalar), then more
            emit_mask(0)
            emit_mask(1)
            for kc in range(KC):
                sl = slice(kc * 128, (kc + 1) * 128)
                if kc % 2 == 0:
                    nc.vector.tensor_copy(h[0][:, sl], gcn_v[:, sl])
                else:
                    nc.scalar.copy(h[0][:, sl], gcn_v[:, sl])
            for l in range(2, 12):
                emit_mask(l)

            den = cpool.tile([128, 1], f32, tag="den")
            nc.vector.tensor_reduce(den[:], adjR_v, mybir.AxisListType.X,
                                    Alu.add)
            nc.vector.tensor_scalar_add(den[:], den[:], 1.0)
            recip = cpool.tile([128, 1], f32, tag="recip")
            nc.vector.reciprocal(recip[:], den[:])

            # identity for the MLP transposes (gpsimd, idle at start)
            identity = cpool.tile([128, 128], f16, tag="ident")
            make_identity(nc, identity[:])

            # -------- GCN layers --------------------------------------------
            Act = mybir.ActivationFunctionType

            def emit_s(ly, p):
                """s-phase for label pair p: one N=256 matmul per kc.
                psum AND sbuf tile share layout [q, kc, l, i] so the
                psum->sbuf cast is contiguous; split into two halves on
                vector + scalar (gpsimd has no PSUM access)."""
                ps = spsum.tile([128, KC, 2, 128], f32, tag="spsum",
                                name="spsum")
                for kc in range(KC):
                    nc.tensor.matmul(
                        ps[:, kc, :, :],
                        lhsT=h[ly][:, kc * 128:(kc + 1) * 128],
                        rhs=maskT[:, 2 * p:2 * p + 2, :],
                        start=True, stop=True,
                    )
                sT = sT_pool.tile([128, KC, 2, 128], f16, tag="sT", name="sT")
                nc.vector.tensor_copy(sT[:, 0:2], ps[:, 0:2])
                nc.scalar.copy(sT[:, 2:4], ps[:, 2:4])
                return sT

            def get_w(ly, p):
                """Weight pair p: resident slice or streamed tile."""
                if 2 * p + 1 < R_RES:
                    return wres[:, 2 * p:2 * p + 2]
                w = wst_pool.tile([128, 2, KC, D], f16, tag="wst", name="wst")
                eng = nc.sync if p % 2 == 0 else nc.gpsimd
                eng.dma_start(w[:], wT_e[:, 2 * p:2 * p + 2])
                return w

            for ly in range(NUM_LAYERS):
                pm = mpsum.tile([128, D], f32, tag="mm", name="mm")
                sT_q = [emit_s(ly, 0), emit_s(ly, 1)]
                for p in range(NP):
                    if ly == 0 and 2 * (p + 6) < L:
                        emit_mask(2 * (p + 6))
                        emit_mask(2 * (p + 6) + 1)
                    if p + 2 < NP:
                        sT_q.append(emit_s(ly, p + 2))
                    w = get_w(ly, p)
                    sT = sT_q[p]
                    for kc in range(KC):
                        for l2 in range(2):
                            i = p * 2 * KC + kc * 2 + l2
                            nc.tensor.matmul(
                                pm[:],
                                lhsT=sT[:, kc, l2, :],
                                rhs=w[:, l2, kc, :],
                                start=(i == 0), stop=(i == L * KC - 1),
                            )
                if ly == 0:
                    # MLP weights: load during layer 2 (slack window)
                    mlpw_sb = cpool.tile([128, 2, KC, D], f16, tag="mlpw")
                    nc.sync.dma_start(mlpw_sb[:], mlpw_e)
                # relu(msg * recip) -> next h (fp16), chunked per kc
                for kc in range(KC):
                    sl = slice(kc * 128, (kc + 1) * 128)
                    if kc % 2 == 0:
                        nc.vector.tensor_scalar(h[ly + 1][:, sl], pm[:, sl],
                                                recip[:], 0.0,
                                                Alu.mult, Alu.max)
                    else:
                        nc.scalar.activation(h[ly + 1][:, sl], pm[:, sl],
                                             Act.Relu, scale=recip[:])

            # -------- MLP ---------------------------------------------------
            w0T_v = mlpw_sb[:, 0]
            w1T_v = mlpw_sb[:, 1]
            h_own = h[NUM_LAYERS]
            hT = cpool.tile([128, KC, 128], f16, tag="hT")
            pt = mpsum.tile([128, KC, 128], f16, tag="mm", name="ptr")
            for kc in range(KC):
                nc.tensor.transpose(pt[:, kc, :],
                                    h_own[:, kc * 128:(kc + 1) * 128],
                                    identity[:])
            nc.vector.tensor_copy(hT[:, 0:2], pt[:, 0:2])
            nc.scalar.copy(hT[:, 2:4], pt[:, 2:4])

            x1T = cpool.tile([128, KC, 128], f16, tag="x1T")
            px1 = mpsum.tile([128, KC, 128], f32, tag="mm", name="px1")
            for blk in range(KC):
                for kc in range(KC):
                    nc.tensor.matmul(
                        px1[:, blk, :],
                        lhsT=w0T_v[:, kc, blk * 128:(blk + 1) * 128],
                        rhs=hT[:, kc, :],
                        start=(kc == 0), stop=(kc == KC - 1),
                    )
            for blk in range(KC):
                if blk % 2 == 0:
                    nc.vector.tensor_scalar(x1T[:, blk, :], px1[:, blk, :],
                                            b0_v[:, blk:blk + 1], 0.0,
                                            Alu.add, Alu.max)
                else:
                    nc.scalar.activation(x1T[:, blk, :], px1[:, blk, :],
                                         Act.Relu, bias=b0_v[:, blk:blk + 1])

            x2 = cpool.tile([128, KC, 128], f32, tag="x2")
            px2 = mpsum.tile([128, KC, 128], f32, tag="mm", name="px2")
            for blk in range(KC):
                for kc in range(KC):
                    nc.tensor.matmul(
                        px2[:, blk, :],
                        lhsT=w1T_v[:, kc, blk * 128:(blk + 1) * 128],
                        rhs=x1T[:, kc, :],
                        start=(kc == 0), stop=(kc == KC - 1),
                    )
            for blk in range(KC):
                if blk % 2 == 0:
                    nc.vector.tensor_scalar(x2[:, blk, :], px2[:, blk, :],
                                            b1_v[:, blk:blk + 1], 0.0,
                                            Alu.add, Alu.max)
                else:
                    nc.scalar.activation(x2[:, blk, :], px2[:, blk, :],
                                         Act.Relu, bias=b1_v[:, blk:blk + 1])

            nc.sync.dma_start(out_e, x2[:])

    nc.compile()
    return nc


def _get_nc():
    if "nc" not in _CACHE:
        _CACHE["nc"] = _build_nc()
    return _CACHE["nc"]


def kernel(gcn_inputs, word_seq_len, adj_matrix, dep_label_matrix,
           w_params, mlp_w0, mlp_b0, mlp_w1, mlp_b1, **_unused):
    from concourse.bass_utils import run_bass_kernel_spmd

    gcn = np.asarray(gcn_inputs, dtype=np.float32)
    adj = np.asarray(adj_matrix, dtype=np.float32)
    lab = np.asarray(dep_label_matrix)
    w = np.asarray(w_params, dtype=np.float32)
    w0 = np.asarray(mlp_w0, dtype=np.float32)
    w1 = np.asarray(mlp_w1, dtype=np.float32)
    b0 = np.asarray(mlp_b0, dtype=np.float32)
    b1 = np.asarray(mlp_b1, dtype=np.float32)

    # wT[kmod, l, kc, d] = w[l, d, kc*128+kmod]  (shared by all cores)
    wT = w.transpose(0, 2, 1).reshape(L, KC, 128, D).transpose(2, 0, 1, 3)
    wT = np.ascontiguousarray(wT).astype(np.float16)
    w0T = w0.T.reshape(KC, 128, D).transpose(1, 0, 2)
    w1T = w1.T.reshape(KC, 128, D).transpose(1, 0, 2)
    mlpw = np.ascontiguousarray(
        np.stack([w0T, w1T], axis=1)).astype(np.float16)   # [128, 2, KC, D]
    b0r = b0.reshape(KC, 128).T                            # [128, KC]
    b1r = b1.reshape(KC, 128).T
    labf = lab.astype(np.float32)

    in_maps = []
    for c in range(NCORES):
        packc = np.empty((N, P_TOT), dtype=np.float32)
        packc[:, P_GCN:P_GCN + D] = gcn[c]
        packc[:, P_ADJT:P_ADJT + N] = adj[c].T
        packc[:, P_LABT:P_LABT + N] = labf[c].T
        packc[:, P_ADJR:P_ADJR + N] = adj[c]
        packc[:, P_B0:P_B0 + KC] = b0r
        packc[:, P_B1:P_B1 + KC] = b1r
        in_maps.append({
            "pack": packc,
            "wT": wT,
            "mlpw": mlpw,
        })

    nc = _get_nc()
    res = run_bass_kernel_spmd(nc, in_maps, list(range(NCORES)))

    out = np.empty((B, N, D), dtype=np.float32)
    for c in range(NCORES):
        arr = res.results[c]["out"]          # [dmod, dblk, i]
        out[c] = np.transpose(arr, (2, 1, 0)).reshape(N, D)
    return out
